# revision 11
# baseline (speedup 1.0000x reference)
"""DCPNet rigid-alignment head on 8 Trainium2 NeuronCores.

Data-parallel over batch: B=16 samples -> 2 per core. Per sample the device
computes, in one fused pipeline:
  pd[m,n]  = ||se_n||^2 - 2 te_m . se_n + ||te_m||^2   (one PE accumulation:
             4 K-chunks of the embedding matmul + 1 augmented K=2 matmul that
             adds -0.5*xx[n] and -0.5*yy[m]; pd = -2 * psum)
  d        = sqrt(pd)                       (ACT Sqrt directly on PSUM)
  E        = exp(-d)                        (unnormalized softmax weights)
  C[n,:]   = [sum_m E[m,n]*tgt_m | sum_m E[m,n]]   (PE matmul with ones col)
  corr     = C[:,0:3] / C[:,3]              (soft correspondences)
  out44    = 4x4 moment matrix [H_raw, N*src_mean; N*corr_mean, N]
The host does only the per-sample 3x3 SVD -> R, t, euler angles.

All big matmuls run in bf16 (the PE streams 1 column/cycle for 16-bit moving
operands vs 2 cycles for fp32r, so this halves PE time; fp32 PSUM
accumulation keeps pd accurate to ~0.3 absolute out of ~1000, i.e. d to
~5e-3 out of ~32, well inside the softmax tolerance). The sqrt runs as a
single ACT pass per score tile; the exp phase is batched globally after all
sqrts so the kernel pays exactly one sqrt->exp table switch. Elementwise
casts/squares are split across the DVE and Pool engines, which are otherwise
idle. Set _ACT_MODE = "ln" to fall back to the 3-pass ln/exp/exp pipeline
(single table set) if the sqrt table is too coarse.
"""

import sys

if "/opt/trn_rl_repo" not in sys.path:
    sys.path.insert(0, "/opt/trn_rl_repo")

import numpy as np

_B, _N, _D = 16, 1024, 512
_NCORES = 8
_SPC = _B // _NCORES  # samples per core

_KC = _D // 128  # 4 contraction chunks
_MC = _N // 128  # 8 partition chunks of the score matrix
_NH = _N // 512  # 2 free-dim halves

_ACT_MODE = "sqrt"  # "sqrt" (2 ACT passes + 1 table switch) or "ln" (3 passes)

_state = {}


def _build():
    # NOTE: walrus's LDW-dedup (--enable-ldw-opt) is incompatible with the
    # fast-weight-load path that bf16 stationary operands trigger, so this
    # kernel keeps the default (dedup off); FWL already halves LDWEIGHTS cost.
    if "nc" in _state:
        return _state["nc"]

    from contextlib import ExitStack

    import concourse.tile as tile
    from concourse import bacc, mybir
    from concourse.masks import make_identity

    fp32 = mybir.dt.float32
    f32r = mybir.dt.float32r
    bf16 = mybir.dt.bfloat16
    AF = mybir.ActivationFunctionType

    nc = bacc.Bacc()
    srcs = nc.declare_dram_parameter("srcs", [_SPC, 3, _N], fp32, isOutput=False)
    tgts = nc.declare_dram_parameter("tgts", [_SPC, 3, _N], fp32, isOutput=False)
    semb = nc.declare_dram_parameter("srcs_emb", [_SPC, _D, _N], fp32, isOutput=False)
    temb = nc.declare_dram_parameter("tgts_emb", [_SPC, _D, _N], fp32, isOutput=False)
    out44 = nc.declare_dram_parameter("out44", [_SPC, 4, 4], fp32, isOutput=True)

    with ExitStack() as ctx:
        tc = ctx.enter_context(tile.TileContext(nc))
        singles = ctx.enter_context(tc.tile_pool(name="singles", bufs=1))
        stage = ctx.enter_context(tc.tile_pool(name="stage", bufs=4))
        emb = ctx.enter_context(tc.tile_pool(name="emb", bufs=2))
        sqp = ctx.enter_context(tc.tile_pool(name="sqp", bufs=4))
        dpool = ctx.enter_context(tc.tile_pool(name="dpool", bufs=2))
        epool = ctx.enter_context(tc.tile_pool(name="epool", bufs=3))
        small = ctx.enter_context(tc.tile_pool(name="small", bufs=2))
        # PSUM budget (8 banks): g2 2 banks x 2 bufs, c2 2 banks x 1,
        # small psums 1 bank x 2.
        psg = ctx.enter_context(tc.tile_pool(name="psg", bufs=2, space="PSUM"))
        psc = ctx.enter_context(tc.tile_pool(name="psc", bufs=1, space="PSUM"))
        pss = ctx.enter_context(tc.tile_pool(name="pss", bufs=2, space="PSUM"))

        ident = singles.tile([4, 4], fp32)
        make_identity(nc, ident)
        # yy reduction: -0.5 weights into output row 0 (M=1)
        neghalf = singles.tile([128, 1], bf16)
        nc.vector.memset(neghalf, -0.5)
        # xx reduction: M=2, col0 = 0 -> row0 unused, col1 = -0.5 -> row1 =
        # -0.5*xx.  Row 1 lands on PSUM partition 1, so the copy into the
        # augmented rhs row (also partition 1) never crosses partitions.
        znh = singles.tile([128, 2], bf16)
        nc.vector.memset(znh[:, 0:1], 0.0)
        nc.vector.memset(znh[:, 1:2], -0.5)

        # ---- per-sample persistent tiles ----
        sef = [[None] * _KC for _ in range(_SPC)]
        tef = [[None] * _KC for _ in range(_SPC)]
        se_bf, te_bf, d_t, aug_lhsT, aug_rhs, sT, tTf, tT, c2 = (
            [None] * _SPC for _ in range(9)
        )
        for s in range(_SPC):
            for k in range(_KC):
                sef[s][k] = stage.tile([128, _N], fp32, tag="sef", name=f"sef{s}{k}")
                tef[s][k] = stage.tile([128, _N], fp32, tag="tef", name=f"tef{s}{k}")
            se_bf[s] = emb.tile([128, _KC, _N], bf16, tag="se", name=f"se{s}")
            te_bf[s] = emb.tile([128, _KC, _N], bf16, tag="te", name=f"te{s}")
            d_t[s] = dpool.tile([128, _MC, _N], fp32, tag="d", name=f"d{s}")
            aug_lhsT[s] = small.tile([2, _N], bf16, tag="auglhs", name=f"al{s}")
            aug_rhs[s] = small.tile([2, _N], bf16, tag="augrhs", name=f"ar{s}")
            # transposed point tiles: [128, q, d]; d=3 data cols + ones col
            sT[s] = small.tile([128, _MC, 4], f32r, tag="sT", name=f"sT{s}")
            tTf[s] = small.tile([128, _MC, 4], fp32, tag="tTf", name=f"tTf{s}")
            tT[s] = small.tile([128, _MC, 4], bf16, tag="tT", name=f"tT{s}")
            # DVE APs must start at partition 0, so memset both rows; the
            # data rows get overwritten by the reduction copies below.
            nc.vector.memset(aug_lhsT[s], 1.0)
            nc.vector.memset(sT[s].bitcast(fp32)[:, :, 3], 1.0)
            nc.vector.memset(tT[s][:, :, 3], 1.0)

        # ---- DMA issue: sync ring carries se + the point gathers, scalar
        # ring carries te.  Per-k pieces so compute starts on chunk arrival.
        for s in range(_SPC):
            se_src = semb[s].rearrange("(k p) n -> p k n", p=128)
            for k in range(_KC):
                nc.sync.dma_start(out=sef[s][k], in_=se_src[:, k, :])
        for s in range(_SPC):
            te_src = temb[s].rearrange("(k p) n -> p k n", p=128)
            for k in range(_KC):
                nc.scalar.dma_start(out=tef[s][k], in_=te_src[:, k, :])
        for s in range(_SPC):
            srcs_nd = srcs[s].rearrange("d n -> n d").bitcast(f32r)
            tgts_nd = tgts[s].rearrange("d n -> n d")
            for q in range(_MC):
                nc.sync.dma_start(
                    out=sT[s][:, q, 0:3],
                    in_=srcs_nd[q * 128 : (q + 1) * 128, :],
                )
                nc.sync.dma_start(
                    out=tTf[s][:, q, 0:3],
                    in_=tgts_nd[q * 128 : (q + 1) * 128, :],
                )

        def phase1a(s):
            """Casts + squares: se on DVE, te on Pool (both otherwise idle)."""
            sqv = [None] * _KC
            sqg = [None] * _KC
            for k in range(_KC):
                nc.vector.tensor_copy(se_bf[s][:, k, :], sef[s][k])
                nc.gpsimd.tensor_copy(te_bf[s][:, k, :], tef[s][k])
                sqv[k] = sqp.tile([128, _N], bf16, tag="sqv", name=f"sqv{s}{k}")
                sqg[k] = sqp.tile([128, _N], bf16, tag="sqg", name=f"sqg{s}{k}")
                nc.vector.tensor_mul(sqv[k], se_bf[s][:, k, :], se_bf[s][:, k, :])
                nc.gpsimd.tensor_mul(sqg[k], te_bf[s][:, k, :], te_bf[s][:, k, :])
            return sqv, sqg

        def phase1b(s, sqv, sqg):
            """-0.5*xx / -0.5*yy reductions on the PE, then the row copies.

            The te reductions go first, k-interleaved with DMA arrival; the
            se reductions then run back-to-back (their squares are all ready
            by the time the last te chunk lands).  The two pairs share the
            pss ring sequentially so only two psum banks are ever live.
            """
            redy = [
                pss.tile([1, 512], fp32, tag="ps1", name=f"ry{s}{h}")
                for h in range(_NH)
            ]
            for k in range(_KC):
                for h in range(_NH):
                    hs = slice(h * 512, (h + 1) * 512)
                    nc.tensor.matmul(
                        redy[h], neghalf, sqg[k][:, hs],
                        start=(k == 0), stop=(k == _KC - 1),
                    )
            for h in range(_NH):
                # -0.5*yy -> aug_lhsT row 0
                nc.vector.tensor_copy(
                    aug_lhsT[s][0:1, h * 512 : (h + 1) * 512], redy[h]
                )
            redx = [
                pss.tile([2, 512], fp32, tag="ps1", name=f"rx{s}{h}")
                for h in range(_NH)
            ]
            for k in range(_KC):
                for h in range(_NH):
                    hs = slice(h * 512, (h + 1) * 512)
                    nc.tensor.matmul(
                        redx[h], znh, sqv[k][:, hs], start=(k == 0), stop=(k == _KC - 1)
                    )
            for h in range(_NH):
                # copy both psum rows (row0 = zeros, row1 = -0.5*xx) with a
                # partition-0-based AP, then restore the ones row.
                nc.vector.tensor_copy(
                    aug_rhs[s][:, h * 512 : (h + 1) * 512], redx[h]
                )
            nc.vector.memset(aug_rhs[s][0:1, :], 1.0)

        def score_chunk(s, m):
            """One m-chunk of the score matrix: matmuls + activation pass 1."""
            msl = slice(m * 128, (m + 1) * 128)
            g2 = psg.tile([128, _NH, 512], fp32, tag="g2", name=f"g2_{s}{m}")
            for k in range(_KC):
                for nh in range(_NH):
                    nc.tensor.matmul(
                        g2[:, nh, :],
                        te_bf[s][:, k, msl],
                        se_bf[s][:, k, nh * 512 : (nh + 1) * 512],
                        start=(k == 0),
                        stop=False,
                    )
            for nh in range(_NH):
                nc.tensor.matmul(
                    g2[:, nh, :],
                    aug_lhsT[s][:, msl],
                    aug_rhs[s][:, nh * 512 : (nh + 1) * 512],
                    start=False,
                    stop=True,
                )
            gflat = g2.rearrange("p a b -> p (a b)")
            if _ACT_MODE == "sqrt":
                # d = sqrt(-2 * psum), one ACT pass per tile
                nc.scalar.activation(
                    out=d_t[s][:, m, :], in_=gflat, func=AF.Sqrt, scale=-2.0
                )
            else:
                # t = ln(-2 * psum); the exp phase below applies exp(0.5 t)
                nc.scalar.activation(
                    out=d_t[s][:, m, :], in_=gflat, func=AF.Ln, scale=-2.0
                )

        def exp_pair(s, j):
            """E = exp(-d) for chunk pair j, then its four E-matmuls."""
            psl = slice(2 * j, 2 * j + 2)
            dd = d_t[s][:, psl, :].rearrange("p a b -> p (a b)")
            if _ACT_MODE == "ln":
                nc.scalar.activation(out=dd, in_=dd, func=AF.Exp, scale=0.5)
            e2 = epool.tile([128, 2, _N], bf16, tag="e", name=f"e{s}{j}")
            nc.scalar.activation(
                out=e2.rearrange("p a b -> p (a b)"), in_=dd, func=AF.Exp, scale=-1.0
            )
            for i in range(2):
                m = 2 * j + i
                for nh in range(_NH):
                    nc.tensor.matmul(
                        c2[s][:, nh, :],
                        tT[s][:, m, :],
                        e2[:, i, nh * 512 : (nh + 1) * 512],
                        start=(m == 0),
                        stop=(m == _MC - 1),
                    )

        def tail(s):
            """Normalize, moment matrix, store."""
            c_sb = small.tile([4, _NH, 512], fp32, tag="csb", name=f"csb{s}")
            nc.vector.tensor_copy(c_sb, c2[s])
            corr_all = small.tile([128, _MC, 4], f32r, tag="corr", name=f"corr{s}")
            nc.vector.memset(corr_all.bitcast(fp32), 1.0)
            c_flat = c_sb.rearrange("p a b -> p (a b)")
            for q in range(_MC):
                ct_ps = pss.tile([128, 4], fp32, tag="ps1", name=f"ct{s}{q}")
                nc.tensor.transpose(ct_ps, c_flat[:, q * 128 : (q + 1) * 128], ident)
                rs = small.tile([128, 1], fp32, tag="rs", name=f"rs{s}{q}")
                nc.vector.reciprocal(rs, ct_ps[:, 3:4])
                nc.vector.tensor_scalar(
                    out=corr_all[:, q, 0:3],
                    in0=ct_ps[:, 0:3],
                    scalar1=rs,
                    scalar2=None,
                    op0=mybir.AluOpType.mult,
                )
            o_ps = pss.tile([4, 4], fp32, tag="ps1", name=f"o{s}")
            for q in range(_MC):
                nc.tensor.matmul(
                    o_ps,
                    sT[s][:, q, :],
                    corr_all[:, q, :],
                    start=(q == 0),
                    stop=(q == _MC - 1),
                )
            o_sb = small.tile([4, 4], fp32, tag="osb", name=f"ot{s}")
            nc.vector.tensor_copy(o_sb, o_ps)
            nc.sync.dma_start(out=out44[s], in_=o_sb)

        # ---- emission: reductions and score chunks interleaved so each
        # engine's FIFO matches data arrival; the exp phase runs globally
        # after all sqrts (one table switch).
        c2[0] = psc.tile([4, _NH, 512], fp32, tag="c2", name="c2_0")
        c2[1] = psc.tile([4, _NH, 512], fp32, tag="c2", name="c2_1")

        sq0 = phase1a(0)
        phase1b(0, *sq0)
        sq1 = phase1a(1)
        for m in range(_MC):
            score_chunk(0, m)
        phase1b(1, *sq1)
        for m in range(_MC):
            score_chunk(1, m)
        # tT casts sit late in the DVE FIFO so the (slow) point gathers never
        # block the phase-1 row copies; they finish long before the E-matmuls.
        for s in range(_SPC):
            nc.vector.tensor_copy(tT[s][:, :, 0:3], tTf[s][:, :, 0:3])
        for s in range(_SPC):
            for j in range(_MC // 2):
                exp_pair(s, j)
            tail(s)

    nc.finalize()
    _state["nc"] = nc
    return nc


def _postprocess(o44):
    """o44: [B, 4, 4] moment matrices -> [B, 6] (euler angles, translation)."""
    o = o44.astype(np.float64)
    H_raw = o[:, 0:3, 0:3]
    ssum = o[:, 0:3, 3]
    csum = o[:, 3, 0:3]
    cnt = o[:, 3, 3][:, None, None]
    H = H_raw - ssum[:, :, None] * csum[:, None, :] / cnt
    u, _, vh = np.linalg.svd(H)
    v = np.swapaxes(vh, -1, -2)
    r = v @ np.swapaxes(u, -1, -2)
    det = np.linalg.det(r)
    flip = np.where(det[:, None] < 0, np.array([1.0, 1.0, -1.0]), 1.0)
    v = v * flip[:, None, :]
    R = v @ np.swapaxes(u, -1, -2)
    sm = ssum / cnt[:, :, 0]
    cm = csum / cnt[:, :, 0]
    t = -np.einsum("bij,bj->bi", R, sm) + cm
    cy = np.sqrt(R[:, 2, 2] ** 2 + R[:, 1, 2] ** 2)
    ax = np.arctan2(-R[:, 1, 2], R[:, 2, 2])
    ay = np.arctan2(R[:, 0, 2], cy)
    az = np.arctan2(-R[:, 0, 1], R[:, 0, 0])
    return np.concatenate([np.stack([ax, ay, az], 1), t], axis=1).astype(np.float32)


def kernel(srcs, tgts, srcs_emb, tgts_emb, **run_kwargs):
    from concourse.bass_utils import run_bass_kernel_spmd

    nc = _build()
    in_maps = []
    for c in range(_NCORES):
        sl = slice(c * _SPC, (c + 1) * _SPC)
        in_maps.append(
            {
                "srcs": np.ascontiguousarray(srcs[sl], dtype=np.float32),
                "tgts": np.ascontiguousarray(tgts[sl], dtype=np.float32),
                "srcs_emb": np.ascontiguousarray(srcs_emb[sl], dtype=np.float32),
                "tgts_emb": np.ascontiguousarray(tgts_emb[sl], dtype=np.float32),
            }
        )
    res = run_bass_kernel_spmd(nc, in_maps, list(range(_NCORES)), **run_kwargs)
    o44 = np.concatenate(
        [np.asarray(res.results[c]["out44"]) for c in range(_NCORES)], axis=0
    )
    out = _postprocess(o44)
    if run_kwargs:
        _state["last_results"] = res
    return out


# revision 17
# speedup vs baseline: 1.2248x; 1.2248x over previous
"""DCPNet rigid-alignment head on 8 Trainium2 NeuronCores.

Data-parallel over batch: B=16 samples -> 2 per core. Per sample the device
computes, in one fused pipeline:
  pd[m,n]  = ||se_n||^2 - 2 te_m . se_n + ||te_m||^2   (one PE accumulation:
             4 K-chunks of the embedding matmul + 1 augmented K=2 matmul that
             adds -0.5*xx[n] and -0.5*yy[m]; pd = -2 * psum)
  d        = sqrt(pd)                       (ACT Sqrt directly on PSUM)
  E        = exp(-d)                        (unnormalized softmax weights)
  C[n,:]   = [sum_m E[m,n]*tgt_m | sum_m E[m,n]]   (PE matmul with ones col)
  corr     = C[:,0:3] / C[:,3]              (soft correspondences)
  out44    = 4x4 moment matrix [H_raw, N*src_mean; N*corr_mean, N]
The host does only the per-sample 3x3 SVD -> R, t, euler angles.

All big matmuls run in bf16 (the PE streams 1 column/cycle for 16-bit moving
operands vs 2 cycles for fp32r, so this halves PE time; fp32 PSUM
accumulation keeps pd accurate to ~0.3 absolute out of ~1000, i.e. d to
~5e-3 out of ~32, well inside the softmax tolerance). The sqrt runs as a
single ACT pass per score tile; the exp phase is batched globally after all
sqrts so the kernel pays exactly one sqrt->exp table switch. Elementwise
casts/squares are split across the DVE and Pool engines, which are otherwise
idle. Set _ACT_MODE = "ln" to fall back to the 3-pass ln/exp/exp pipeline
(single table set) if the sqrt table is too coarse.
"""

import sys

if "/opt/trn_rl_repo" not in sys.path:
    sys.path.insert(0, "/opt/trn_rl_repo")

import numpy as np

_B, _N, _D = 16, 1024, 512
_NCORES = 8
_SPC = _B // _NCORES  # samples per core

_KC = _D // 128  # 4 contraction chunks
_MC = _N // 128  # 8 partition chunks of the score matrix
_NH = _N // 512  # 2 free-dim halves

_ACT_MODE = "ln"  # "sqrt" (2 ACT passes + table switches) or "ln" (3 passes)

_state = {}


def _patch_act_tables():
    """Make natural_log_exp_and_others the only set providing Ln/Exp/Square.
    The table-load inserter runs after the Tile scheduler, so the scheduler
    freely interleaves ACT functions; with one set there is exactly one
    ACT_TABLE_LOAD no matter how the ops interleave."""
    from concourse import bacc, hw_specs, mybir

    if getattr(bacc, "_dcp_act_patch", False):
        return
    orig = hw_specs.get_activation_tables

    def patched(module_arch):
        tables = dict(orig(module_arch))
        used = {
            mybir.ActivationFunctionType.Ln,
            mybir.ActivationFunctionType.Exp,
            mybir.ActivationFunctionType.Square,
        }
        for name, funcs in tables.items():
            if name != "natural_log_exp_and_others":
                funcs.difference_update(used)
        return tables

    bacc.get_activation_tables = patched
    bacc._dcp_act_patch = True


def _build():
    # NOTE: walrus's LDW-dedup (--enable-ldw-opt) is incompatible with the
    # fast-weight-load path that bf16 stationary operands trigger, so this
    # kernel keeps the default (dedup off); FWL already halves LDWEIGHTS cost.
    if "nc" in _state:
        return _state["nc"]

    from contextlib import ExitStack

    import concourse.tile as tile
    from concourse import bacc, mybir
    from concourse.masks import make_identity

    _patch_act_tables()

    fp32 = mybir.dt.float32
    f32r = mybir.dt.float32r
    bf16 = mybir.dt.bfloat16
    AF = mybir.ActivationFunctionType

    nc = bacc.Bacc()
    srcs = nc.declare_dram_parameter("srcs", [_SPC, 3, _N], fp32, isOutput=False)
    tgts = nc.declare_dram_parameter("tgts", [_SPC, 3, _N], fp32, isOutput=False)
    semb = nc.declare_dram_parameter("srcs_emb", [_SPC, _D, _N], fp32, isOutput=False)
    temb = nc.declare_dram_parameter("tgts_emb", [_SPC, _D, _N], fp32, isOutput=False)
    out44 = nc.declare_dram_parameter("out44", [_SPC, 4, 4], fp32, isOutput=True)

    with ExitStack() as ctx:
        tc = ctx.enter_context(tile.TileContext(nc))
        singles = ctx.enter_context(tc.tile_pool(name="singles", bufs=1))
        stage = ctx.enter_context(tc.tile_pool(name="stage", bufs=4))
        emb = ctx.enter_context(tc.tile_pool(name="emb", bufs=2))
        sqp = ctx.enter_context(tc.tile_pool(name="sqp", bufs=4))
        dpool = ctx.enter_context(tc.tile_pool(name="dpool", bufs=2))
        epool = ctx.enter_context(tc.tile_pool(name="epool", bufs=3))
        small = ctx.enter_context(tc.tile_pool(name="small", bufs=2))
        # PSUM budget (8 banks): g2 2 banks x 2 bufs, c2 2 banks x 1,
        # small psums 1 bank x 2.
        psg = ctx.enter_context(tc.tile_pool(name="psg", bufs=2, space="PSUM"))
        psc = ctx.enter_context(tc.tile_pool(name="psc", bufs=1, space="PSUM"))
        pss = ctx.enter_context(tc.tile_pool(name="pss", bufs=2, space="PSUM"))

        ident = singles.tile([4, 4], fp32)
        make_identity(nc, ident)
        # yy reduction: -0.5 weights into output row 0 (M=1)
        neghalf = singles.tile([128, 1], bf16)
        nc.vector.memset(neghalf, -0.5)
        # xx reduction: M=2, col0 = 0 -> row0 unused, col1 = -0.5 -> row1 =
        # -0.5*xx.  Row 1 lands on PSUM partition 1, so the copy into the
        # augmented rhs row (also partition 1) never crosses partitions.
        znh = singles.tile([128, 2], bf16)
        nc.vector.memset(znh[:, 0:1], 0.0)
        nc.vector.memset(znh[:, 1:2], -0.5)

        # ---- per-sample persistent tiles ----
        sef = [[None] * _KC for _ in range(_SPC)]
        tef = [[None] * _KC for _ in range(_SPC)]
        se_bf, te_bf, d_t, aug_lhsT, aug_rhs, sT, tTf, tT, c2 = (
            [None] * _SPC for _ in range(9)
        )
        for s in range(_SPC):
            for k in range(_KC):
                sef[s][k] = stage.tile([128, _N], fp32, tag="sef", name=f"sef{s}{k}")
                tef[s][k] = stage.tile([128, _N], fp32, tag="tef", name=f"tef{s}{k}")
            se_bf[s] = emb.tile([128, _KC, _N], bf16, tag="se", name=f"se{s}")
            te_bf[s] = emb.tile([128, _KC, _N], bf16, tag="te", name=f"te{s}")
            d_t[s] = dpool.tile([128, _MC, _N], fp32, tag="d", name=f"d{s}")
            aug_lhsT[s] = small.tile([2, _N], bf16, tag="auglhs", name=f"al{s}")
            aug_rhs[s] = small.tile([2, _N], bf16, tag="augrhs", name=f"ar{s}")
            # transposed point tiles: [128, q, d]; d=3 data cols + ones col
            sT[s] = small.tile([128, _MC, 4], f32r, tag="sT", name=f"sT{s}")
            tTf[s] = small.tile([128, _MC, 4], fp32, tag="tTf", name=f"tTf{s}")
            tT[s] = small.tile([128, _MC, 4], bf16, tag="tT", name=f"tT{s}")
            # DVE APs must start at partition 0, so memset both rows; the
            # data rows get overwritten by the reduction copies below.
            nc.vector.memset(aug_lhsT[s], 1.0)
            nc.vector.memset(sT[s].bitcast(fp32)[:, :, 3], 1.0)
            nc.vector.memset(tT[s][:, :, 3], 1.0)

        # ---- DMA issue: sync ring carries se + the point gathers, scalar
        # ring carries te.  Per-k pieces so compute starts on chunk arrival.
        for s in range(_SPC):
            se_src = semb[s].rearrange("(k p) n -> p k n", p=128)
            for k in range(_KC):
                nc.sync.dma_start(out=sef[s][k], in_=se_src[:, k, :])
        for s in range(_SPC):
            te_src = temb[s].rearrange("(k p) n -> p k n", p=128)
            for k in range(_KC):
                nc.scalar.dma_start(out=tef[s][k], in_=te_src[:, k, :])
        # gathers after the bulk issues (issue cost ~0.9us each on the sync
        # engine would otherwise delay the bulk transfers); tgts first since
        # the E-matmuls need them ~15us before the tails need srcs.
        for s in range(_SPC):
            tgts_nd = tgts[s].rearrange("d n -> n d")
            for q in range(_MC):
                nc.sync.dma_start(
                    out=tTf[s][:, q, 0:3],
                    in_=tgts_nd[q * 128 : (q + 1) * 128, :],
                )
        for s in range(_SPC):
            srcs_nd = srcs[s].rearrange("d n -> n d").bitcast(f32r)
            for q in range(_MC):
                nc.sync.dma_start(
                    out=sT[s][:, q, 0:3],
                    in_=srcs_nd[q * 128 : (q + 1) * 128, :],
                )

        def phase1a(s):
            """Casts + squares.  The Pool engine is slow (~2-4us per big
            tile), so it only gets the te squares; both casts run on the DVE,
            and the se squares ride the ACT engine for sample 0 (ACT is idle
            until the first Ln) and the DVE for sample 1.  Squares read the
            fp32 staging tiles directly so they never wait on the casts."""
            sqv = [None] * _KC
            sqg = [None] * _KC
            for k in range(_KC):
                nc.vector.tensor_copy(se_bf[s][:, k, :], sef[s][k])
                nc.vector.tensor_copy(te_bf[s][:, k, :], tef[s][k])
                sqv[k] = sqp.tile([128, _N], bf16, tag="sqv", name=f"sqv{s}{k}")
                sqg[k] = sqp.tile([128, _N], bf16, tag="sqg", name=f"sqg{s}{k}")
                if s == 0:
                    nc.scalar.activation(out=sqv[k], in_=sef[s][k], func=AF.Square)
                else:
                    nc.vector.tensor_mul(sqv[k], sef[s][k], sef[s][k])
                nc.gpsimd.tensor_mul(sqg[k], tef[s][k], tef[s][k])
            return sqv, sqg

        def phase1b(s, sqv, sqg):
            """-0.5*xx / -0.5*yy reductions on the PE, then the row copies.

            The te reductions go first, k-interleaved with DMA arrival; the
            se reductions then run back-to-back (their squares are all ready
            by the time the last te chunk lands).  The two pairs share the
            pss ring sequentially so only two psum banks are ever live.
            """
            redy = [
                pss.tile([1, 512], fp32, tag="ps1", name=f"ry{s}{h}")
                for h in range(_NH)
            ]
            for k in range(_KC):
                for h in range(_NH):
                    hs = slice(h * 512, (h + 1) * 512)
                    nc.tensor.matmul(
                        redy[h], neghalf, sqg[k][:, hs],
                        start=(k == 0), stop=(k == _KC - 1),
                    )
            for h in range(_NH):
                # -0.5*yy -> aug_lhsT row 0 (Pool cannot read PSUM)
                nc.vector.tensor_copy(
                    aug_lhsT[s][0:1, h * 512 : (h + 1) * 512], redy[h]
                )
            redx = [
                pss.tile([2, 512], fp32, tag="ps1", name=f"rx{s}{h}")
                for h in range(_NH)
            ]
            for k in range(_KC):
                for h in range(_NH):
                    hs = slice(h * 512, (h + 1) * 512)
                    nc.tensor.matmul(
                        redx[h], znh, sqv[k][:, hs], start=(k == 0), stop=(k == _KC - 1)
                    )
            for h in range(_NH):
                # copy both psum rows (row0 = zeros, row1 = -0.5*xx) with a
                # partition-0-based AP, then restore the ones row.
                nc.vector.tensor_copy(
                    aug_rhs[s][:, h * 512 : (h + 1) * 512], redx[h]
                )
            nc.vector.memset(aug_rhs[s][0:1, :], 1.0)

        def score_chunk(s, m):
            """One m-chunk of the score matrix: matmuls + activation pass 1."""
            msl = slice(m * 128, (m + 1) * 128)
            g2 = psg.tile([128, _NH, 512], fp32, tag="g2", name=f"g2_{s}{m}")
            for k in range(_KC):
                for nh in range(_NH):
                    nc.tensor.matmul(
                        g2[:, nh, :],
                        te_bf[s][:, k, msl],
                        se_bf[s][:, k, nh * 512 : (nh + 1) * 512],
                        start=(k == 0),
                        stop=False,
                    )
            for nh in range(_NH):
                nc.tensor.matmul(
                    g2[:, nh, :],
                    aug_lhsT[s][:, msl],
                    aug_rhs[s][:, nh * 512 : (nh + 1) * 512],
                    start=False,
                    stop=True,
                )
            gflat = g2.rearrange("p a b -> p (a b)")
            if _ACT_MODE == "sqrt":
                # d = sqrt(-2 * psum), one ACT pass per tile
                nc.scalar.activation(
                    out=d_t[s][:, m, :], in_=gflat, func=AF.Sqrt, scale=-2.0
                )
            else:
                # t = ln(-2 * psum); the exp phase below applies exp(0.5 t)
                nc.scalar.activation(
                    out=d_t[s][:, m, :], in_=gflat, func=AF.Ln, scale=-2.0
                )

        def exp_pair(s, j):
            """E = exp(-d) for chunk pair j, then its four E-matmuls."""
            psl = slice(2 * j, 2 * j + 2)
            dd = d_t[s][:, psl, :].rearrange("p a b -> p (a b)")
            if _ACT_MODE == "ln":
                nc.scalar.activation(out=dd, in_=dd, func=AF.Exp, scale=0.5)
            e2 = epool.tile([128, 2, _N], bf16, tag="e", name=f"e{s}{j}")
            nc.scalar.activation(
                out=e2.rearrange("p a b -> p (a b)"), in_=dd, func=AF.Exp, scale=-1.0
            )
            for i in range(2):
                m = 2 * j + i
                for nh in range(_NH):
                    nc.tensor.matmul(
                        c2[s][:, nh, :],
                        tT[s][:, m, :],
                        e2[:, i, nh * 512 : (nh + 1) * 512],
                        start=(m == 0),
                        stop=(m == _MC - 1),
                    )

        def tail(s):
            """Normalize, moment matrix, store."""
            c_sb = small.tile([4, _NH, 512], fp32, tag="csb", name=f"csb{s}")
            nc.vector.tensor_copy(c_sb, c2[s])
            corr_all = small.tile([128, _MC, 4], f32r, tag="corr", name=f"corr{s}")
            nc.vector.memset(corr_all.bitcast(fp32), 1.0)
            c_flat = c_sb.rearrange("p a b -> p (a b)")
            for q in range(_MC):
                ct_ps = pss.tile([128, 4], fp32, tag="ps1", name=f"ct{s}{q}")
                nc.tensor.transpose(ct_ps, c_flat[:, q * 128 : (q + 1) * 128], ident)
                rs = small.tile([128, 1], fp32, tag="rs", name=f"rs{s}{q}")
                nc.vector.reciprocal(rs, ct_ps[:, 3:4])
                nc.vector.tensor_scalar(
                    out=corr_all[:, q, 0:3],
                    in0=ct_ps[:, 0:3],
                    scalar1=rs,
                    scalar2=None,
                    op0=mybir.AluOpType.mult,
                )
            o_ps = pss.tile([4, 4], fp32, tag="ps1", name=f"o{s}")
            for q in range(_MC):
                nc.tensor.matmul(
                    o_ps,
                    sT[s][:, q, :],
                    corr_all[:, q, :],
                    start=(q == 0),
                    stop=(q == _MC - 1),
                )
            o_sb = small.tile([4, 4], fp32, tag="osb", name=f"ot{s}")
            nc.vector.tensor_copy(o_sb, o_ps)
            nc.sync.dma_start(out=out44[s], in_=o_sb)

        # ---- emission: reductions and score chunks interleaved so each
        # engine's FIFO matches data arrival; the exp phase runs globally
        # after all sqrts (one table switch).
        c2[0] = psc.tile([4, _NH, 512], fp32, tag="c2", name="c2_0")
        c2[1] = psc.tile([4, _NH, 512], fp32, tag="c2", name="c2_1")

        sq0 = phase1a(0)
        phase1b(0, *sq0)
        sq1 = phase1a(1)
        for m in range(_MC):
            score_chunk(0, m)
        phase1b(1, *sq1)
        for m in range(_MC):
            score_chunk(1, m)
        # tT casts sit late in the DVE FIFO so the (slow) point gathers never
        # block the phase-1 row copies; they finish long before the E-matmuls.
        for s in range(_SPC):
            nc.vector.tensor_copy(tT[s][:, :, 0:3], tTf[s][:, :, 0:3])
        for s in range(_SPC):
            for j in range(_MC // 2):
                exp_pair(s, j)
            tail(s)

    nc.finalize()
    _state["nc"] = nc
    return nc


def _postprocess(o44):
    """o44: [B, 4, 4] moment matrices -> [B, 6] (euler angles, translation)."""
    o = o44.astype(np.float64)
    H_raw = o[:, 0:3, 0:3]
    ssum = o[:, 0:3, 3]
    csum = o[:, 3, 0:3]
    cnt = o[:, 3, 3][:, None, None]
    H = H_raw - ssum[:, :, None] * csum[:, None, :] / cnt
    u, _, vh = np.linalg.svd(H)
    v = np.swapaxes(vh, -1, -2)
    r = v @ np.swapaxes(u, -1, -2)
    det = np.linalg.det(r)
    flip = np.where(det[:, None] < 0, np.array([1.0, 1.0, -1.0]), 1.0)
    v = v * flip[:, None, :]
    R = v @ np.swapaxes(u, -1, -2)
    sm = ssum / cnt[:, :, 0]
    cm = csum / cnt[:, :, 0]
    t = -np.einsum("bij,bj->bi", R, sm) + cm
    cy = np.sqrt(R[:, 2, 2] ** 2 + R[:, 1, 2] ** 2)
    ax = np.arctan2(-R[:, 1, 2], R[:, 2, 2])
    ay = np.arctan2(R[:, 0, 2], cy)
    az = np.arctan2(-R[:, 0, 1], R[:, 0, 0])
    return np.concatenate([np.stack([ax, ay, az], 1), t], axis=1).astype(np.float32)


def kernel(srcs, tgts, srcs_emb, tgts_emb, **run_kwargs):
    from concourse.bass_utils import run_bass_kernel_spmd

    nc = _build()
    in_maps = []
    for c in range(_NCORES):
        sl = slice(c * _SPC, (c + 1) * _SPC)
        in_maps.append(
            {
                "srcs": np.ascontiguousarray(srcs[sl], dtype=np.float32),
                "tgts": np.ascontiguousarray(tgts[sl], dtype=np.float32),
                "srcs_emb": np.ascontiguousarray(srcs_emb[sl], dtype=np.float32),
                "tgts_emb": np.ascontiguousarray(tgts_emb[sl], dtype=np.float32),
            }
        )
    res = run_bass_kernel_spmd(nc, in_maps, list(range(_NCORES)), **run_kwargs)
    o44 = np.concatenate(
        [np.asarray(res.results[c]["out44"]) for c in range(_NCORES)], axis=0
    )
    out = _postprocess(o44)
    if run_kwargs:
        _state["last_results"] = res
    return out


# revision 20
# speedup vs baseline: 1.2376x; 1.0104x over previous
"""DCPNet rigid-alignment head on 8 Trainium2 NeuronCores.

Data-parallel over batch: B=16 samples -> 2 per core. Per sample the device
computes, in one fused pipeline:
  pd[m,n]  = ||se_n||^2 - 2 te_m . se_n + ||te_m||^2   (one PE accumulation:
             4 K-chunks of the embedding matmul + 1 augmented K=2 matmul that
             adds -0.5*xx[n] and -0.5*yy[m]; pd = -2 * psum)
  d        = sqrt(pd)                       (ACT Sqrt directly on PSUM)
  E        = exp(-d)                        (unnormalized softmax weights)
  C[n,:]   = [sum_m E[m,n]*tgt_m | sum_m E[m,n]]   (PE matmul with ones col)
  corr     = C[:,0:3] / C[:,3]              (soft correspondences)
  out44    = 4x4 moment matrix [H_raw, N*src_mean; N*corr_mean, N]
The host does only the per-sample 3x3 SVD -> R, t, euler angles.

All big matmuls run in bf16 (the PE streams 1 column/cycle for 16-bit moving
operands vs 2 cycles for fp32r, so this halves PE time; fp32 PSUM
accumulation keeps pd accurate to ~0.3 absolute out of ~1000, i.e. d to
~5e-3 out of ~32, well inside the softmax tolerance). The sqrt runs as a
single ACT pass per score tile; the exp phase is batched globally after all
sqrts so the kernel pays exactly one sqrt->exp table switch. Elementwise
casts/squares are split across the DVE and Pool engines, which are otherwise
idle. Set _ACT_MODE = "ln" to fall back to the 3-pass ln/exp/exp pipeline
(single table set) if the sqrt table is too coarse.
"""

import sys

if "/opt/trn_rl_repo" not in sys.path:
    sys.path.insert(0, "/opt/trn_rl_repo")

import numpy as np

_B, _N, _D = 16, 1024, 512
_NCORES = 8
_SPC = _B // _NCORES  # samples per core

_KC = _D // 128  # 4 contraction chunks
_MC = _N // 128  # 8 partition chunks of the score matrix
_NH = _N // 512  # 2 free-dim halves

_ACT_MODE = "ln"  # "sqrt" (2 ACT passes + table switches) or "ln" (3 passes)

_state = {}


def _patch_act_tables():
    """Make natural_log_exp_and_others the only set providing Ln/Exp/Square.
    The table-load inserter runs after the Tile scheduler, so the scheduler
    freely interleaves ACT functions; with one set there is exactly one
    ACT_TABLE_LOAD no matter how the ops interleave."""
    from concourse import bacc, hw_specs, mybir

    if getattr(bacc, "_dcp_act_patch", False):
        return
    orig = hw_specs.get_activation_tables

    def patched(module_arch):
        tables = dict(orig(module_arch))
        used = {
            mybir.ActivationFunctionType.Ln,
            mybir.ActivationFunctionType.Exp,
            mybir.ActivationFunctionType.Square,
        }
        for name, funcs in tables.items():
            if name != "natural_log_exp_and_others":
                funcs.difference_update(used)
        return tables

    bacc.get_activation_tables = patched
    bacc._dcp_act_patch = True


def _build():
    # NOTE: walrus's LDW-dedup (--enable-ldw-opt) is incompatible with the
    # fast-weight-load path that bf16 stationary operands trigger, so this
    # kernel keeps the default (dedup off); FWL already halves LDWEIGHTS cost.
    if "nc" in _state:
        return _state["nc"]

    from contextlib import ExitStack

    import concourse.tile as tile
    from concourse import bacc, mybir
    from concourse.masks import make_identity

    _patch_act_tables()

    fp32 = mybir.dt.float32
    f32r = mybir.dt.float32r
    bf16 = mybir.dt.bfloat16
    AF = mybir.ActivationFunctionType

    nc = bacc.Bacc()
    srcs = nc.declare_dram_parameter("srcs", [_SPC, 3, _N], fp32, isOutput=False)
    tgts = nc.declare_dram_parameter("tgts", [_SPC, 3, _N], fp32, isOutput=False)
    semb = nc.declare_dram_parameter("srcs_emb", [_SPC, _D, _N], fp32, isOutput=False)
    temb = nc.declare_dram_parameter("tgts_emb", [_SPC, _D, _N], fp32, isOutput=False)
    out44 = nc.declare_dram_parameter("out44", [_SPC, 4, 4], fp32, isOutput=True)

    with ExitStack() as ctx:
        tc = ctx.enter_context(tile.TileContext(nc))
        singles = ctx.enter_context(tc.tile_pool(name="singles", bufs=1))
        stage = ctx.enter_context(tc.tile_pool(name="stage", bufs=4))
        emb = ctx.enter_context(tc.tile_pool(name="emb", bufs=2))
        sqp = ctx.enter_context(tc.tile_pool(name="sqp", bufs=4))
        dpool = ctx.enter_context(tc.tile_pool(name="dpool", bufs=2))
        epool = ctx.enter_context(tc.tile_pool(name="epool", bufs=3))
        small = ctx.enter_context(tc.tile_pool(name="small", bufs=2))
        # PSUM budget (8 banks): g2 2 banks x 2 bufs, c2 2 banks x 1,
        # small psums 1 bank x 2.
        psg = ctx.enter_context(tc.tile_pool(name="psg", bufs=2, space="PSUM"))
        psc = ctx.enter_context(tc.tile_pool(name="psc", bufs=1, space="PSUM"))
        pss = ctx.enter_context(tc.tile_pool(name="pss", bufs=2, space="PSUM"))

        ident = singles.tile([4, 4], fp32)
        make_identity(nc, ident)
        # yy reduction: -0.5 weights into output row 0 (M=1)
        neghalf = singles.tile([128, 1], bf16)
        nc.vector.memset(neghalf, -0.5)
        # xx reduction: M=2, col0 = 0 -> row0 unused, col1 = -0.5 -> row1 =
        # -0.5*xx.  Row 1 lands on PSUM partition 1, so the copy into the
        # augmented rhs row (also partition 1) never crosses partitions.
        znh = singles.tile([128, 2], bf16)
        nc.vector.memset(znh[:, 0:1], 0.0)
        nc.vector.memset(znh[:, 1:2], -0.5)

        # ---- per-sample persistent tiles ----
        sef = [[None] * _KC for _ in range(_SPC)]
        tef = [[None] * _KC for _ in range(_SPC)]
        se_bf, te_bf, d_t, aug_lhsT, aug_rhs, sT, tTf, tT, c2 = (
            [None] * _SPC for _ in range(9)
        )
        for s in range(_SPC):
            for k in range(_KC):
                sef[s][k] = stage.tile([128, _N], fp32, tag="sef", name=f"sef{s}{k}")
                tef[s][k] = stage.tile([128, _N], fp32, tag="tef", name=f"tef{s}{k}")
            se_bf[s] = emb.tile([128, _KC, _N], bf16, tag="se", name=f"se{s}")
            te_bf[s] = emb.tile([128, _KC, _N], bf16, tag="te", name=f"te{s}")
            d_t[s] = dpool.tile([128, _MC, _N], fp32, tag="d", name=f"d{s}")
            aug_lhsT[s] = small.tile([2, _N], bf16, tag="auglhs", name=f"al{s}")
            aug_rhs[s] = small.tile([2, _N], bf16, tag="augrhs", name=f"ar{s}")
            # transposed point tiles: [128, q, d]; d=3 data cols + ones col
            sT[s] = small.tile([128, _MC, 4], f32r, tag="sT", name=f"sT{s}")
            tTf[s] = small.tile([128, _MC, 4], fp32, tag="tTf", name=f"tTf{s}")
            tT[s] = small.tile([128, _MC, 4], bf16, tag="tT", name=f"tT{s}")
            # DVE APs must start at partition 0, so memset both rows; the
            # data rows get overwritten by the reduction copies below.
            nc.vector.memset(aug_lhsT[s], 1.0)
            nc.vector.memset(sT[s].bitcast(fp32)[:, :, 3], 1.0)
            nc.vector.memset(tT[s][:, :, 3], 1.0)

        # ---- DMA issue: sync ring carries se + the point gathers, scalar
        # ring carries te.  Per-k pieces so compute starts on chunk arrival.
        for s in range(_SPC):
            se_src = semb[s].rearrange("(k p) n -> p k n", p=128)
            for k in range(_KC):
                nc.sync.dma_start(out=sef[s][k], in_=se_src[:, k, :])
        for s in range(_SPC):
            te_src = temb[s].rearrange("(k p) n -> p k n", p=128)
            for k in range(_KC):
                nc.scalar.dma_start(out=tef[s][k], in_=te_src[:, k, :])
        # gathers after the bulk issues (issue cost ~0.9us each on the sync
        # engine would otherwise delay the bulk transfers); tgts first since
        # the E-matmuls need them ~15us before the tails need srcs.
        for s in range(_SPC):
            tgts_nd = tgts[s].rearrange("d n -> n d")
            for q in range(_MC):
                nc.sync.dma_start(
                    out=tTf[s][:, q, 0:3],
                    in_=tgts_nd[q * 128 : (q + 1) * 128, :],
                )
        for s in range(_SPC):
            srcs_nd = srcs[s].rearrange("d n -> n d").bitcast(f32r)
            for q in range(_MC):
                nc.sync.dma_start(
                    out=sT[s][:, q, 0:3],
                    in_=srcs_nd[q * 128 : (q + 1) * 128, :],
                )

        def phase1a(s):
            """Casts + squares.  The Pool engine is slow (~2-4us per big
            tile), so it only gets the te squares; both casts run on the DVE,
            and the se squares ride the ACT engine for sample 0 (ACT is idle
            until the first Ln) and the DVE for sample 1.  Squares read the
            fp32 staging tiles directly so they never wait on the casts."""
            sqv = [None] * _KC
            sqg = [None] * _KC
            for k in range(_KC):
                nc.vector.tensor_copy(se_bf[s][:, k, :], sef[s][k])
                nc.vector.tensor_copy(te_bf[s][:, k, :], tef[s][k])
                sqv[k] = sqp.tile([128, _N], bf16, tag="sqv", name=f"sqv{s}{k}")
                sqg[k] = sqp.tile([128, _N], bf16, tag="sqg", name=f"sqg{s}{k}")
                nc.vector.tensor_mul(sqv[k], sef[s][k], sef[s][k])
                nc.gpsimd.tensor_mul(sqg[k], tef[s][k], tef[s][k])
            return sqv, sqg

        def phase1b(s, sqv, sqg):
            """-0.5*xx / -0.5*yy reductions on the PE, then the row copies.

            The te reductions go first, k-interleaved with DMA arrival; the
            se reductions then run back-to-back (their squares are all ready
            by the time the last te chunk lands).  The two pairs share the
            pss ring sequentially so only two psum banks are ever live.
            """
            redy = [
                pss.tile([1, 512], fp32, tag="ps1", name=f"ry{s}{h}")
                for h in range(_NH)
            ]
            for k in range(_KC):
                for h in range(_NH):
                    hs = slice(h * 512, (h + 1) * 512)
                    nc.tensor.matmul(
                        redy[h], neghalf, sqg[k][:, hs],
                        start=(k == 0), stop=(k == _KC - 1),
                    )
            for h in range(_NH):
                # -0.5*yy -> aug_lhsT row 0 (Pool cannot read PSUM)
                nc.vector.tensor_copy(
                    aug_lhsT[s][0:1, h * 512 : (h + 1) * 512], redy[h]
                )
            redx = [
                pss.tile([2, 512], fp32, tag="ps1", name=f"rx{s}{h}")
                for h in range(_NH)
            ]
            for k in range(_KC):
                for h in range(_NH):
                    hs = slice(h * 512, (h + 1) * 512)
                    nc.tensor.matmul(
                        redx[h], znh, sqv[k][:, hs], start=(k == 0), stop=(k == _KC - 1)
                    )
            for h in range(_NH):
                # copy both psum rows (row0 = zeros, row1 = -0.5*xx) with a
                # partition-0-based AP, then restore the ones row.
                nc.vector.tensor_copy(
                    aug_rhs[s][:, h * 512 : (h + 1) * 512], redx[h]
                )
            nc.vector.memset(aug_rhs[s][0:1, :], 1.0)

        def score_chunk(s, m):
            """One m-chunk of the score matrix: matmuls + activation pass 1."""
            msl = slice(m * 128, (m + 1) * 128)
            g2 = psg.tile([128, _NH, 512], fp32, tag="g2", name=f"g2_{s}{m}")
            for k in range(_KC):
                for nh in range(_NH):
                    nc.tensor.matmul(
                        g2[:, nh, :],
                        te_bf[s][:, k, msl],
                        se_bf[s][:, k, nh * 512 : (nh + 1) * 512],
                        start=(k == 0),
                        stop=False,
                    )
            for nh in range(_NH):
                nc.tensor.matmul(
                    g2[:, nh, :],
                    aug_lhsT[s][:, msl],
                    aug_rhs[s][:, nh * 512 : (nh + 1) * 512],
                    start=False,
                    stop=True,
                )
            gflat = g2.rearrange("p a b -> p (a b)")
            if _ACT_MODE == "sqrt":
                # d = sqrt(-2 * psum), one ACT pass per tile
                nc.scalar.activation(
                    out=d_t[s][:, m, :], in_=gflat, func=AF.Sqrt, scale=-2.0
                )
            else:
                # t = ln(-2 * psum); the exp phase below applies exp(0.5 t)
                nc.scalar.activation(
                    out=d_t[s][:, m, :], in_=gflat, func=AF.Ln, scale=-2.0
                )

        def exp_quad(s, j):
            """E = exp(-d) for a 4-chunk batch, then its eight E-matmuls.
            Big ACTIVATEs amortize the ~350-cycle fixed cost per op."""
            psl = slice(4 * j, 4 * j + 4)
            dd = d_t[s][:, psl, :].rearrange("p a b -> p (a b)")
            if _ACT_MODE == "ln":
                nc.scalar.activation(out=dd, in_=dd, func=AF.Exp, scale=0.5)
            e4 = epool.tile([128, 4, _N], bf16, tag="e", name=f"e{s}{j}")
            nc.scalar.activation(
                out=e4.rearrange("p a b -> p (a b)"), in_=dd, func=AF.Exp, scale=-1.0
            )
            for i in range(4):
                m = 4 * j + i
                for nh in range(_NH):
                    nc.tensor.matmul(
                        c2[s][:, nh, :],
                        tT[s][:, m, :],
                        e4[:, i, nh * 512 : (nh + 1) * 512],
                        start=(m == 0),
                        stop=(m == _MC - 1),
                    )

        def tail(s):
            """Normalize, moment matrix, store."""
            c_sb = small.tile([4, _NH, 512], fp32, tag="csb", name=f"csb{s}")
            nc.vector.tensor_copy(c_sb, c2[s])
            corr_all = small.tile([128, _MC, 4], f32r, tag="corr", name=f"corr{s}")
            nc.vector.memset(corr_all.bitcast(fp32), 1.0)
            c_flat = c_sb.rearrange("p a b -> p (a b)")
            for q in range(_MC):
                ct_ps = pss.tile([128, 4], fp32, tag="ps1", name=f"ct{s}{q}")
                nc.tensor.transpose(ct_ps, c_flat[:, q * 128 : (q + 1) * 128], ident)
                rs = small.tile([128, 1], fp32, tag="rs", name=f"rs{s}{q}")
                nc.vector.reciprocal(rs, ct_ps[:, 3:4])
                nc.vector.tensor_scalar(
                    out=corr_all[:, q, 0:3],
                    in0=ct_ps[:, 0:3],
                    scalar1=rs,
                    scalar2=None,
                    op0=mybir.AluOpType.mult,
                )
            o_ps = pss.tile([4, 4], fp32, tag="ps1", name=f"o{s}")
            for q in range(_MC):
                nc.tensor.matmul(
                    o_ps,
                    sT[s][:, q, :],
                    corr_all[:, q, :],
                    start=(q == 0),
                    stop=(q == _MC - 1),
                )
            o_sb = small.tile([4, 4], fp32, tag="osb", name=f"ot{s}")
            nc.vector.tensor_copy(o_sb, o_ps)
            nc.sync.dma_start(out=out44[s], in_=o_sb)

        # ---- emission: reductions and score chunks interleaved so each
        # engine's FIFO matches data arrival; the exp phase runs globally
        # after all sqrts (one table switch).
        c2[0] = psc.tile([4, _NH, 512], fp32, tag="c2", name="c2_0")
        c2[1] = psc.tile([4, _NH, 512], fp32, tag="c2", name="c2_1")

        sq0 = phase1a(0)
        phase1b(0, *sq0)
        sq1 = phase1a(1)
        for m in range(_MC):
            score_chunk(0, m)
        phase1b(1, *sq1)
        for m in range(_MC):
            score_chunk(1, m)
        # tT casts sit late in the DVE FIFO so the (slow) point gathers never
        # block the phase-1 row copies; they finish long before the E-matmuls.
        for s in range(_SPC):
            nc.vector.tensor_copy(tT[s][:, :, 0:3], tTf[s][:, :, 0:3])
        for s in range(_SPC):
            for j in range(_MC // 4):
                exp_quad(s, j)
            tail(s)

    nc.finalize()
    _state["nc"] = nc
    return nc


def _postprocess(o44):
    """o44: [B, 4, 4] moment matrices -> [B, 6] (euler angles, translation)."""
    o = o44.astype(np.float64)
    H_raw = o[:, 0:3, 0:3]
    ssum = o[:, 0:3, 3]
    csum = o[:, 3, 0:3]
    cnt = o[:, 3, 3][:, None, None]
    H = H_raw - ssum[:, :, None] * csum[:, None, :] / cnt
    u, _, vh = np.linalg.svd(H)
    v = np.swapaxes(vh, -1, -2)
    r = v @ np.swapaxes(u, -1, -2)
    det = np.linalg.det(r)
    flip = np.where(det[:, None] < 0, np.array([1.0, 1.0, -1.0]), 1.0)
    v = v * flip[:, None, :]
    R = v @ np.swapaxes(u, -1, -2)
    sm = ssum / cnt[:, :, 0]
    cm = csum / cnt[:, :, 0]
    t = -np.einsum("bij,bj->bi", R, sm) + cm
    cy = np.sqrt(R[:, 2, 2] ** 2 + R[:, 1, 2] ** 2)
    ax = np.arctan2(-R[:, 1, 2], R[:, 2, 2])
    ay = np.arctan2(R[:, 0, 2], cy)
    az = np.arctan2(-R[:, 0, 1], R[:, 0, 0])
    return np.concatenate([np.stack([ax, ay, az], 1), t], axis=1).astype(np.float32)


def kernel(srcs, tgts, srcs_emb, tgts_emb, **run_kwargs):
    from concourse.bass_utils import run_bass_kernel_spmd

    nc = _build()
    in_maps = []
    for c in range(_NCORES):
        sl = slice(c * _SPC, (c + 1) * _SPC)
        in_maps.append(
            {
                "srcs": np.ascontiguousarray(srcs[sl], dtype=np.float32),
                "tgts": np.ascontiguousarray(tgts[sl], dtype=np.float32),
                "srcs_emb": np.ascontiguousarray(srcs_emb[sl], dtype=np.float32),
                "tgts_emb": np.ascontiguousarray(tgts_emb[sl], dtype=np.float32),
            }
        )
    res = run_bass_kernel_spmd(nc, in_maps, list(range(_NCORES)), **run_kwargs)
    o44 = np.concatenate(
        [np.asarray(res.results[c]["out44"]) for c in range(_NCORES)], axis=0
    )
    out = _postprocess(o44)
    if run_kwargs:
        _state["last_results"] = res
    return out
